# revision 8
# baseline (speedup 1.0000x reference)
"""RNN-T Joiner kernel for 8 Trainium2 NeuronCores.

Reference computation (per batch element n):
    enc = encoder_out[n] @ W_enc.T + b_enc          # (T=200, J=512)
    dec = decoder_out[n] @ W_dec.T + b_dec          # (U=50,  J=512)
    x   = tanh(enc[:,None,:] + dec[None,:,:])       # (T, U, J)
    out = x @ W_out.T + b_out                       # (T, U, V=500)

Sharding: data-parallel over N=8 (one batch element per core).

Device-side layout (everything j/c-major so no on-chip transposes needed):
    encT[j, t]  = projection computed directly in transposed form
    XT[j, tu]   = tanh(encT[:,t] + decT[:,u]) built with a broadcast add
    logits tile = XT_tile.T @ W_outT  (PE, float32r fast path), + b_out on DVE
"""

import numpy as np

N, T, U = 8, 200, 50
C = 512   # enc/dec feature dim
J = 512   # joint dim
V = 500   # vocab
TU = T * U
P = 128
KC = J // P          # 4 contraction chunks of 128
T_BLK = 64           # t's per block (64*50 = 3200 tu = 25 M-tiles of 128)
GROUP = 5            # M-tiles per output DMA group

_CACHE = {}


def _build_bass(main_dtype_name="float32r"):
    import concourse.bass as bass
    import concourse.mybir as mybir
    import concourse.tile as tile
    from concourse import bacc

    main_dt = getattr(mybir.dt, main_dtype_name)
    f32 = mybir.dt.float32

    nc = bacc.Bacc("TRN2", target_bir_lowering=False, debug=False, num_devices=N)

    # DRAM I/O (per-core views; weights replicated across cores)
    enc_in = nc.dram_tensor("enc_in", [C, T], f32, kind="ExternalInput").ap()
    dec_in = nc.dram_tensor("dec_in", [C, U], f32, kind="ExternalInput").ap()
    w_enc = nc.dram_tensor("w_enc", [C, J], f32, kind="ExternalInput").ap()
    w_dec = nc.dram_tensor("w_dec", [C, J], f32, kind="ExternalInput").ap()
    w_out = nc.dram_tensor("w_out", [J, V], main_dt, kind="ExternalInput").ap()
    b_enc = nc.dram_tensor("b_enc", [P, KC], f32, kind="ExternalInput").ap()
    b_dec = nc.dram_tensor("b_dec", [P, KC], f32, kind="ExternalInput").ap()
    b_out = nc.dram_tensor("b_out", [P, V], f32, kind="ExternalInput").ap()
    logits = nc.dram_tensor("logits", [TU, V], f32, kind="ExternalOutput").ap()

    with tile.TileContext(nc) as tc:
        with (
            tc.tile_pool(name="const", bufs=1) as const,
            tc.tile_pool(name="xt", bufs=8) as xtp,
            tc.tile_pool(name="lout", bufs=3) as lp,
            tc.tile_pool(name="ps", bufs=6, space="PSUM") as psp,
        ):
            # ---- load weights + inputs -------------------------------------
            w_enc_sb = const.tile([P, KC, J], f32)
            nc.sync.dma_start(w_enc_sb[:], w_enc.rearrange("(kc p) j -> p kc j", p=P))
            w_dec_sb = const.tile([P, KC, J], f32)
            nc.sync.dma_start(w_dec_sb[:], w_dec.rearrange("(kc p) j -> p kc j", p=P))
            w_out_sb = const.tile([P, KC, V], main_dt)
            nc.sync.dma_start(w_out_sb[:], w_out.rearrange("(kc p) v -> p kc v", p=P))
            enc_in_sb = const.tile([P, KC, T], f32)
            nc.sync.dma_start(enc_in_sb[:], enc_in.rearrange("(kc p) t -> p kc t", p=P))
            dec_in_sb = const.tile([P, KC, U], f32)
            nc.sync.dma_start(dec_in_sb[:], dec_in.rearrange("(kc p) u -> p kc u", p=P))
            b_enc_sb = const.tile([P, KC], f32)
            nc.sync.dma_start(b_enc_sb[:], b_enc)
            b_dec_sb = const.tile([P, KC], f32)
            nc.sync.dma_start(b_dec_sb[:], b_dec)
            b_out_sb = const.tile([P, V], f32)
            nc.sync.dma_start(b_out_sb[:], b_out)

            # ---- input projections, directly in transposed (j-major) form --
            encT = const.tile([P, KC, T], f32)
            decT = const.tile([P, KC, U], f32)
            for jc in range(KC):
                ps = psp.tile([P, V], f32, tag="ps")
                for kc in range(KC):
                    nc.tensor.matmul(
                        ps[:, :T],
                        lhsT=w_enc_sb[:, kc, jc * P:(jc + 1) * P],
                        rhs=enc_in_sb[:, kc, :],
                        start=(kc == 0),
                        stop=(kc == KC - 1),
                    )
                nc.scalar.activation(
                    encT[:, jc, :], ps[:, :T],
                    mybir.ActivationFunctionType.Identity,
                    bias=b_enc_sb[:, jc:jc + 1],
                )
                ps = psp.tile([P, V], f32, tag="ps")
                for kc in range(KC):
                    nc.tensor.matmul(
                        ps[:, :U],
                        lhsT=w_dec_sb[:, kc, jc * P:(jc + 1) * P],
                        rhs=dec_in_sb[:, kc, :],
                        start=(kc == 0),
                        stop=(kc == KC - 1),
                    )
                nc.scalar.activation(
                    decT[:, jc, :], ps[:, :U],
                    mybir.ActivationFunctionType.Identity,
                    bias=b_dec_sb[:, jc:jc + 1],
                )

            # ---- main loop over t-blocks -----------------------------------
            t0 = 0
            while t0 < T:
                nt = min(T_BLK, T - t0)
                ntu = nt * U
                # XT[j, t, u] = tanh(encT[j, t] + decT[j, u]) for this block
                xts = []
                for kc in range(KC):
                    xt_full = xtp.tile([P, T_BLK, U], main_dt, tag="xt", name=f"xt{kc}")
                    xt = xt_full[:, :nt, :]
                    nc.vector.tensor_add(
                        xt[:],
                        encT[:, kc, t0:t0 + nt, None].to_broadcast((P, nt, U)),
                        decT[:, kc, None, :].to_broadcast((P, nt, U)),
                    )
                    nc.scalar.activation(
                        xt[:], xt[:].bitcast(f32), mybir.ActivationFunctionType.Tanh
                    )
                    xts.append(xt.rearrange("p t u -> p (t u)"))

                # M-tiles of 128 tu rows each, DMA'd out in groups
                tu_base = t0 * U
                n_full = ntu // P
                tail = ntu - n_full * P
                m = 0
                while m < n_full or (m == n_full and tail):
                    g = min(GROUP, n_full - m)  # full tiles in this group
                    take_tail = (m + g == n_full) and tail and (g < GROUP)
                    L = lp.tile([P, GROUP, V], f32, tag="L")
                    for i in range(g + (1 if take_tail else 0)):
                        rows = P if i < g else tail
                        lo = (m + i) * P
                        ps = psp.tile([P, V], f32, tag="ps")
                        for kc in range(KC):
                            nc.tensor.matmul(
                                ps[:rows, :],
                                lhsT=xts[kc][:, lo:lo + rows],
                                rhs=w_out_sb[:, kc, :],
                                start=(kc == 0),
                                stop=(kc == KC - 1),
                            )
                        nc.vector.tensor_add(
                            L[:rows, i, :], ps[:rows, :], b_out_sb[:rows, :]
                        )
                    if g:
                        nc.sync.dma_start(
                            logits[tu_base + m * P: tu_base + (m + g) * P, :]
                            .rearrange("(i p) v -> p i v", p=P),
                            L[:, :g, :],
                        )
                    if take_tail:
                        nc.sync.dma_start(
                            logits[tu_base + n_full * P: tu_base + ntu, :],
                            L[:tail, g, :],
                        )
                        m += 1
                    m += g
                t0 += nt

    nc.compile()
    return nc


def _get_bass():
    if "nc" not in _CACHE:
        _CACHE["nc"] = _build_bass()
    return _CACHE["nc"]


def _pack_inputs(inputs):
    encoder_out = np.ascontiguousarray(
        np.asarray(inputs["encoder_out"], np.float32).transpose(0, 2, 1))
    decoder_out = np.ascontiguousarray(
        np.asarray(inputs["decoder_out"], np.float32).transpose(0, 2, 1))
    WencT = np.ascontiguousarray(np.asarray(inputs["W_enc"], np.float32).T)
    WdecT = np.ascontiguousarray(np.asarray(inputs["W_dec"], np.float32).T)
    WoutT = np.ascontiguousarray(np.asarray(inputs["W_out"], np.float32).T)
    benc = np.ascontiguousarray(
        np.asarray(inputs["b_enc"], np.float32).reshape(KC, P).T)
    bdec = np.ascontiguousarray(
        np.asarray(inputs["b_dec"], np.float32).reshape(KC, P).T)
    bout = np.ascontiguousarray(
        np.broadcast_to(np.asarray(inputs["b_out"], np.float32), (P, V)))
    return [
        {
            "enc_in": encoder_out[n],
            "dec_in": decoder_out[n],
            "w_enc": WencT,
            "w_dec": WdecT,
            "w_out": WoutT,
            "b_enc": benc,
            "b_dec": bdec,
            "b_out": bout,
        }
        for n in range(N)
    ]


def run(inputs, trace=False):
    """Run the bass kernel; returns (output array, BassKernelResults)."""
    from concourse.bass_utils import run_bass_kernel_spmd

    nc = _get_bass()
    in_maps = _pack_inputs(inputs)
    res = run_bass_kernel_spmd(nc, in_maps, core_ids=list(range(N)), trace=trace)
    out = np.stack([r["logits"] for r in res.results])
    return out.reshape(N, T, U, V), res


def kernel(**inputs):
    out, _ = run(inputs)
    return out


# revision 9
# speedup vs baseline: 1.0385x; 1.0385x over previous
"""RNN-T Joiner kernel for 8 Trainium2 NeuronCores.

Reference computation (per batch element n):
    enc = encoder_out[n] @ W_enc.T + b_enc          # (T=200, J=512)
    dec = decoder_out[n] @ W_dec.T + b_dec          # (U=50,  J=512)
    x   = tanh(enc[:,None,:] + dec[None,:,:])       # (T, U, J)
    out = x @ W_out.T + b_out                       # (T, U, V=500)

Sharding: data-parallel over N=8 (one batch element per core).

Device-side dataflow (everything j/c-major, pre-transposed on host):
    PE:     encT/decT projections (fp32), main matmul (bf16, fp32 PSUM)
    GPSIMD: S[j,t,u] = encT[j,t] + decT[j,u]  (broadcast add, bf16 out)
    ACT:    XT = tanh(S)  (bf16)
    DVE:    PSUM -> SBUF evacuation + b_out bias (batched 4 M-tiles/instr)
    DMA:    1MB contiguous output stores
"""

import numpy as np

N, T, U = 8, 200, 50
C = 512   # enc/dec feature dim
J = 512   # joint dim
V = 500   # vocab
TU = T * U
P = 128
KC = J // P          # 4 contraction chunks of 128
T_BLK = 64           # t's per block (64*50 = 3200 tu = 25 M-tiles of 128)
BLK_TU = T_BLK * U   # 3200
GROUP = 4            # M-tiles per PSUM tile / output DMA group

_CACHE = {}


def _build_bass():
    import concourse.bass as bass  # noqa: F401
    import concourse.mybir as mybir
    import concourse.tile as tile
    from concourse import bacc

    bf16 = mybir.dt.bfloat16
    f32 = mybir.dt.float32
    Act = mybir.ActivationFunctionType

    nc = bacc.Bacc("TRN2", target_bir_lowering=False, debug=False, num_devices=N)

    enc_in = nc.dram_tensor("enc_in", [C, T], f32, kind="ExternalInput").ap()
    dec_in = nc.dram_tensor("dec_in", [C, U], f32, kind="ExternalInput").ap()
    w_enc = nc.dram_tensor("w_enc", [C, J], f32, kind="ExternalInput").ap()
    w_dec = nc.dram_tensor("w_dec", [C, J], f32, kind="ExternalInput").ap()
    w_out = nc.dram_tensor("w_out", [J, V], bf16, kind="ExternalInput").ap()
    b_enc = nc.dram_tensor("b_enc", [P, KC], f32, kind="ExternalInput").ap()
    b_dec = nc.dram_tensor("b_dec", [P, KC], f32, kind="ExternalInput").ap()
    b_out = nc.dram_tensor("b_out", [P, V], f32, kind="ExternalInput").ap()
    logits = nc.dram_tensor("logits", [TU, V], f32, kind="ExternalOutput").ap()

    n_full = TU // P          # 78 full M-tiles
    tail = TU - n_full * P    # 16

    with tile.TileContext(nc) as tc:
        with (
            tc.tile_pool(name="const", bufs=1) as const,
            tc.tile_pool(name="s", bufs=3) as sp,
            tc.tile_pool(name="xt", bufs=8) as xtp,
            tc.tile_pool(name="lout", bufs=3) as lp,
            tc.tile_pool(name="ps", bufs=2, space="PSUM") as psp,
        ):
            # ---- load weights + inputs -------------------------------------
            w_enc_sb = const.tile([P, KC, J], f32)
            nc.sync.dma_start(w_enc_sb[:], w_enc.rearrange("(kc p) j -> p kc j", p=P))
            w_dec_sb = const.tile([P, KC, J], f32)
            nc.sync.dma_start(w_dec_sb[:], w_dec.rearrange("(kc p) j -> p kc j", p=P))
            w_out_sb = const.tile([P, KC, V], bf16)
            nc.sync.dma_start(w_out_sb[:], w_out.rearrange("(kc p) v -> p kc v", p=P))
            enc_in_sb = const.tile([P, KC, T], f32)
            nc.sync.dma_start(enc_in_sb[:], enc_in.rearrange("(kc p) t -> p kc t", p=P))
            dec_in_sb = const.tile([P, KC, U], f32)
            nc.sync.dma_start(dec_in_sb[:], dec_in.rearrange("(kc p) u -> p kc u", p=P))
            b_enc_sb = const.tile([P, KC], f32)
            nc.sync.dma_start(b_enc_sb[:], b_enc)
            b_dec_sb = const.tile([P, KC], f32)
            nc.sync.dma_start(b_dec_sb[:], b_dec)
            b_out_sb = const.tile([P, V], f32)
            nc.sync.dma_start(b_out_sb[:], b_out)

            # ---- input projections, directly in transposed (j-major) form --
            encT = const.tile([P, KC, T], f32)
            decT = const.tile([P, KC, U], f32)
            for jc in range(KC):
                ps = psp.tile([P, GROUP, 512], f32, tag="ps", name="pse")
                for kc in range(KC):
                    nc.tensor.matmul(
                        ps[:, 0, :T],
                        lhsT=w_enc_sb[:, kc, jc * P:(jc + 1) * P],
                        rhs=enc_in_sb[:, kc, :],
                        start=(kc == 0),
                        stop=(kc == KC - 1),
                    )
                nc.scalar.activation(
                    encT[:, jc, :], ps[:, 0, :T], Act.Identity,
                    bias=b_enc_sb[:, jc:jc + 1],
                )
                ps = psp.tile([P, GROUP, 512], f32, tag="ps", name="psd")
                for kc in range(KC):
                    nc.tensor.matmul(
                        ps[:, 0, :U],
                        lhsT=w_dec_sb[:, kc, jc * P:(jc + 1) * P],
                        rhs=dec_in_sb[:, kc, :],
                        start=(kc == 0),
                        stop=(kc == KC - 1),
                    )
                nc.scalar.activation(
                    decT[:, jc, :], ps[:, 0, :U], Act.Identity,
                    bias=b_dec_sb[:, jc:jc + 1],
                )

            # ---- XT block production (GPSIMD add -> ACT tanh) --------------
            # xts[block][kc] -> flattened [P, block_tu] bf16 AP
            xts = []
            t0 = 0
            blk = 0
            while t0 < T:
                nt = min(T_BLK, T - t0)
                row = []
                # sub-chunk the first block so matmuls can start early
                nsub = 2 if blk == 0 else 1
                step = nt // nsub
                kc_tiles = []
                for kc in range(KC):
                    s_full = sp.tile([P, T_BLK, U], bf16, tag="s", name=f"s{kc}")
                    x_full = xtp.tile([P, T_BLK, U], bf16, tag="xt", name=f"xt{kc}")
                    kc_tiles.append((s_full[:, :nt, :], x_full[:, :nt, :]))
                for sub in range(nsub):
                    lo, hi = sub * step, (sub + 1) * step
                    for kc in range(KC):
                        s, x = kc_tiles[kc]
                        nc.gpsimd.tensor_add(
                            s[:, lo:hi, :],
                            encT[:, kc, t0 + lo:t0 + hi, None]
                            .to_broadcast((P, hi - lo, U)),
                            decT[:, kc, None, :].to_broadcast((P, hi - lo, U)),
                        )
                        nc.scalar.activation(
                            x[:, lo:hi, :], s[:, lo:hi, :], Act.Tanh
                        )
                for kc in range(KC):
                    row.append(kc_tiles[kc][1].rearrange("p t u -> p (t u)"))
                xts.append(row)
                t0 += nt
                blk += 1

            # ---- main matmul over flat M-tile groups -----------------------
            def lhsT_for(kc, m_lo, rows):
                b = m_lo // BLK_TU
                off = m_lo - b * BLK_TU
                return xts[b][kc][:, off:off + rows]

            m = 0
            while m < n_full:
                g = min(GROUP, n_full - m)
                take_tail = (m + g == n_full) and tail and (g < GROUP)
                nsub = g + (1 if take_tail else 0)
                ps = psp.tile([P, GROUP, 512], f32, tag="ps", name="psm")
                L = lp.tile([P, GROUP, V], f32, tag="L", name="L")
                for i in range(nsub):
                    rows = P if i < g else tail
                    for kc in range(KC):
                        nc.tensor.matmul(
                            ps[:rows, i, :V],
                            lhsT=lhsT_for(kc, (m + i) * P, rows),
                            rhs=w_out_sb[:, kc, :],
                            start=(kc == 0),
                            stop=(kc == KC - 1),
                        )
                nc.vector.tensor_add(
                    L[:, :nsub, :],
                    ps[:, :nsub, :V],
                    b_out_sb[:, None, :].to_broadcast((P, nsub, V)),
                )
                nc.sync.dma_start(
                    logits[m * P:(m + g) * P, :].rearrange("(i p) v -> p i v", p=P),
                    L[:, :g, :],
                )
                if take_tail:
                    nc.sync.dma_start(
                        logits[n_full * P:TU, :],
                        L[:tail, g, :],
                    )
                m += g

    nc.compile()
    return nc


def _get_bass():
    if "nc" not in _CACHE:
        _CACHE["nc"] = _build_bass()
    return _CACHE["nc"]


def _pack_inputs(inputs):
    import ml_dtypes

    encoder_out = np.ascontiguousarray(
        np.asarray(inputs["encoder_out"], np.float32).transpose(0, 2, 1))
    decoder_out = np.ascontiguousarray(
        np.asarray(inputs["decoder_out"], np.float32).transpose(0, 2, 1))
    WencT = np.ascontiguousarray(np.asarray(inputs["W_enc"], np.float32).T)
    WdecT = np.ascontiguousarray(np.asarray(inputs["W_dec"], np.float32).T)
    WoutT = np.ascontiguousarray(
        np.asarray(inputs["W_out"], np.float32).T.astype(ml_dtypes.bfloat16))
    benc = np.ascontiguousarray(
        np.asarray(inputs["b_enc"], np.float32).reshape(KC, P).T)
    bdec = np.ascontiguousarray(
        np.asarray(inputs["b_dec"], np.float32).reshape(KC, P).T)
    bout = np.ascontiguousarray(
        np.broadcast_to(np.asarray(inputs["b_out"], np.float32), (P, V)))
    return [
        {
            "enc_in": encoder_out[n],
            "dec_in": decoder_out[n],
            "w_enc": WencT,
            "w_dec": WdecT,
            "w_out": WoutT,
            "b_enc": benc,
            "b_dec": bdec,
            "b_out": bout,
        }
        for n in range(N)
    ]


def run(inputs, trace=False):
    """Run the bass kernel; returns (output array, BassKernelResults)."""
    from concourse.bass_utils import run_bass_kernel_spmd

    nc = _get_bass()
    in_maps = _pack_inputs(inputs)
    res = run_bass_kernel_spmd(nc, in_maps, core_ids=list(range(N)), trace=trace)
    out = np.stack([r["logits"] for r in res.results])
    return out.reshape(N, T, U, V), res


def kernel(**inputs):
    out, _ = run(inputs)
    return out


# revision 10
# speedup vs baseline: 1.1127x; 1.0715x over previous
"""RNN-T Joiner kernel for 8 Trainium2 NeuronCores.

Reference computation (per batch element n):
    enc = encoder_out[n] @ W_enc.T + b_enc          # (T=200, J=512)
    dec = decoder_out[n] @ W_dec.T + b_dec          # (U=50,  J=512)
    x   = tanh(enc[:,None,:] + dec[None,:,:])       # (T, U, J)
    out = x @ W_out.T + b_out                       # (T, U, V=500)

Sharding: data-parallel over N=8 (one batch element per core).

Device-side dataflow (everything j/c-major, pre-transposed on host):
    PE:     encT/decT projections (fp32), main matmul (bf16, fp32 PSUM)
    GPSIMD: S[j,t,u] = encT[j,t] + decT[j,u]  (broadcast add, bf16 out)
    ACT:    XT = tanh(S)  (bf16)
    DVE:    PSUM -> SBUF evacuation + b_out bias (batched 4 M-tiles/instr)
    DMA:    1MB contiguous output stores
"""

import numpy as np

N, T, U = 8, 200, 50
C = 512   # enc/dec feature dim
J = 512   # joint dim
V = 500   # vocab
TU = T * U
P = 128
KC = J // P          # 4 contraction chunks of 128
T_BLK = 64           # t's per block (64*50 = 3200 tu = 25 M-tiles of 128)
BLK_TU = T_BLK * U   # 3200
GROUP = 4            # M-tiles per PSUM tile / output DMA group

_CACHE = {}


def _build_bass():
    import concourse.bass as bass  # noqa: F401
    import concourse.mybir as mybir
    import concourse.tile as tile
    from concourse import bacc

    bf16 = mybir.dt.bfloat16
    f32 = mybir.dt.float32
    Act = mybir.ActivationFunctionType

    nc = bacc.Bacc("TRN2", target_bir_lowering=False, debug=False, num_devices=N)

    f32r = mybir.dt.float32r
    enc_in = nc.dram_tensor("enc_in", [C, T], f32r, kind="ExternalInput").ap()
    dec_in = nc.dram_tensor("dec_in", [C, U], f32r, kind="ExternalInput").ap()
    w_enc = nc.dram_tensor("w_enc", [C, J], f32r, kind="ExternalInput").ap()
    w_dec = nc.dram_tensor("w_dec", [C, J], f32r, kind="ExternalInput").ap()
    w_out = nc.dram_tensor("w_out", [J, V], bf16, kind="ExternalInput").ap()
    b_enc = nc.dram_tensor("b_enc", [P, KC], f32, kind="ExternalInput").ap()
    b_dec = nc.dram_tensor("b_dec", [P, KC], f32, kind="ExternalInput").ap()
    b_out = nc.dram_tensor("b_out", [P, V], f32, kind="ExternalInput").ap()
    logits = nc.dram_tensor("logits", [TU, V], bf16, kind="ExternalOutput").ap()

    n_full = TU // P          # 78 full M-tiles
    tail = TU - n_full * P    # 16

    with tile.TileContext(nc) as tc:
        with (
            tc.tile_pool(name="const", bufs=1) as const,
            tc.tile_pool(name="s", bufs=3) as sp,
            tc.tile_pool(name="xt", bufs=8) as xtp,
            tc.tile_pool(name="lout", bufs=3) as lp,
            tc.tile_pool(name="ps", bufs=2, space="PSUM") as psp,
        ):
            # ---- load weights + inputs -------------------------------------
            w_enc_sb = const.tile([P, KC, J], f32r)
            w_dec_sb = const.tile([P, KC, J], f32r)
            w_out_sb = const.tile([P, KC, V], bf16)
            enc_in_sb = const.tile([P, KC, T], f32r)
            dec_in_sb = const.tile([P, KC, U], f32r)
            enc_in_r = enc_in.rearrange("(kc p) t -> p kc t", p=P)
            dec_in_r = dec_in.rearrange("(kc p) u -> p kc u", p=P)
            w_enc_r = w_enc.rearrange("(kc p) j -> p kc j", p=P)
            w_dec_r = w_dec.rearrange("(kc p) j -> p kc j", p=P)
            nc.sync.dma_start(enc_in_sb[:], enc_in_r)
            nc.sync.dma_start(dec_in_sb[:], dec_in_r)
            for kc in range(KC):
                nc.sync.dma_start(w_enc_sb[:, kc], w_enc_r[:, kc])
            for kc in range(KC):
                nc.sync.dma_start(w_dec_sb[:, kc], w_dec_r[:, kc])
            nc.sync.dma_start(w_out_sb[:], w_out.rearrange("(kc p) v -> p kc v", p=P))
            b_enc_sb = const.tile([P, KC], f32)
            nc.sync.dma_start(b_enc_sb[:], b_enc)
            b_dec_sb = const.tile([P, KC], f32)
            nc.sync.dma_start(b_dec_sb[:], b_dec)
            b_out_sb = const.tile([P, V], f32)
            nc.sync.dma_start(b_out_sb[:], b_out)

            # ---- input projections, directly in transposed (j-major) form --
            encT = const.tile([P, KC, T], f32)
            decT = const.tile([P, KC, U], f32)
            for jc in range(KC):
                ps = psp.tile([P, GROUP, 512], f32, tag="ps", name="pse")
                for kc in range(KC):
                    nc.tensor.matmul(
                        ps[:, 0, :T],
                        lhsT=w_enc_sb[:, kc, jc * P:(jc + 1) * P],
                        rhs=enc_in_sb[:, kc, :],
                        start=(kc == 0),
                        stop=(kc == KC - 1),
                    )
                nc.scalar.activation(
                    encT[:, jc, :], ps[:, 0, :T], Act.Identity,
                    bias=b_enc_sb[:, jc:jc + 1],
                )
                ps = psp.tile([P, GROUP, 512], f32, tag="ps", name="psd")
                for kc in range(KC):
                    nc.tensor.matmul(
                        ps[:, 0, :U],
                        lhsT=w_dec_sb[:, kc, jc * P:(jc + 1) * P],
                        rhs=dec_in_sb[:, kc, :],
                        start=(kc == 0),
                        stop=(kc == KC - 1),
                    )
                nc.scalar.activation(
                    decT[:, jc, :], ps[:, 0, :U], Act.Identity,
                    bias=b_dec_sb[:, jc:jc + 1],
                )

            # ---- XT block production (GPSIMD add -> ACT tanh) --------------
            # xts[block][kc] -> flattened [P, block_tu] bf16 AP
            xts = []
            t0 = 0
            blk = 0
            while t0 < T:
                nt = min(T_BLK, T - t0)
                row = []
                # sub-chunk the first block so matmuls can start early
                nsub = 4 if blk == 0 else 1
                step = nt // nsub
                kc_tiles = []
                for kc in range(KC):
                    s_full = sp.tile([P, T_BLK, U], bf16, tag="s", name=f"s{kc}")
                    x_full = xtp.tile([P, T_BLK, U], bf16, tag="xt", name=f"xt{kc}")
                    kc_tiles.append((s_full[:, :nt, :], x_full[:, :nt, :]))
                for sub in range(nsub):
                    lo, hi = sub * step, (sub + 1) * step
                    for kc in range(KC):
                        s, x = kc_tiles[kc]
                        eng = (nc.vector if (blk == 0 and kc % 2 == 0)
                               else nc.gpsimd)
                        eng.tensor_add(
                            s[:, lo:hi, :],
                            encT[:, kc, t0 + lo:t0 + hi, None]
                            .to_broadcast((P, hi - lo, U)),
                            decT[:, kc, None, :].to_broadcast((P, hi - lo, U)),
                        )
                        nc.scalar.activation(
                            x[:, lo:hi, :], s[:, lo:hi, :], Act.Tanh
                        )
                for kc in range(KC):
                    row.append(kc_tiles[kc][1].rearrange("p t u -> p (t u)"))
                xts.append(row)
                t0 += nt
                blk += 1

            # ---- main matmul over flat M-tile groups -----------------------
            def lhsT_for(kc, m_lo, rows):
                b = m_lo // BLK_TU
                off = m_lo - b * BLK_TU
                return xts[b][kc][:, off:off + rows]

            m = 0
            while m < n_full:
                g = min(GROUP, n_full - m)
                take_tail = (m + g == n_full) and tail and (g < GROUP)
                nsub = g + (1 if take_tail else 0)
                ps = psp.tile([P, GROUP, 512], f32, tag="ps", name="psm")
                L = lp.tile([P, GROUP, V], bf16, tag="L", name="L")
                for i in range(nsub):
                    rows = P if i < g else tail
                    for kc in range(KC):
                        nc.tensor.matmul(
                            ps[:rows, i, :V],
                            lhsT=lhsT_for(kc, (m + i) * P, rows),
                            rhs=w_out_sb[:, kc, :],
                            start=(kc == 0),
                            stop=(kc == KC - 1),
                        )
                nc.vector.tensor_add(
                    L[:, :nsub, :],
                    ps[:, :nsub, :V],
                    b_out_sb[:, None, :].to_broadcast((P, nsub, V)),
                )
                nc.sync.dma_start(
                    logits[m * P:(m + g) * P, :].rearrange("(i p) v -> p i v", p=P),
                    L[:, :g, :],
                )
                if take_tail:
                    nc.sync.dma_start(
                        logits[n_full * P:TU, :],
                        L[:tail, g, :],
                    )
                m += g

    nc.compile()
    return nc


def _get_bass():
    if "nc" not in _CACHE:
        _CACHE["nc"] = _build_bass()
    return _CACHE["nc"]


def _pack_inputs(inputs):
    import ml_dtypes

    encoder_out = np.ascontiguousarray(
        np.asarray(inputs["encoder_out"], np.float32).transpose(0, 2, 1))
    decoder_out = np.ascontiguousarray(
        np.asarray(inputs["decoder_out"], np.float32).transpose(0, 2, 1))
    WencT = np.ascontiguousarray(np.asarray(inputs["W_enc"], np.float32).T)
    WdecT = np.ascontiguousarray(np.asarray(inputs["W_dec"], np.float32).T)
    WoutT = np.ascontiguousarray(
        np.asarray(inputs["W_out"], np.float32).T.astype(ml_dtypes.bfloat16))
    benc = np.ascontiguousarray(
        np.asarray(inputs["b_enc"], np.float32).reshape(KC, P).T)
    bdec = np.ascontiguousarray(
        np.asarray(inputs["b_dec"], np.float32).reshape(KC, P).T)
    bout = np.ascontiguousarray(
        np.broadcast_to(np.asarray(inputs["b_out"], np.float32), (P, V)))
    return [
        {
            "enc_in": encoder_out[n],
            "dec_in": decoder_out[n],
            "w_enc": WencT,
            "w_dec": WdecT,
            "w_out": WoutT,
            "b_enc": benc,
            "b_dec": bdec,
            "b_out": bout,
        }
        for n in range(N)
    ]


def run(inputs, trace=False):
    """Run the bass kernel; returns (output array, BassKernelResults)."""
    from concourse.bass_utils import run_bass_kernel_spmd

    nc = _get_bass()
    in_maps = _pack_inputs(inputs)
    res = run_bass_kernel_spmd(nc, in_maps, core_ids=list(range(N)), trace=trace)
    out = np.stack([np.asarray(r["logits"], dtype=np.float32)
                    for r in res.results])
    return out.reshape(N, T, U, V), res


def kernel(**inputs):
    out, _ = run(inputs)
    return out


# revision 11
# speedup vs baseline: 1.1354x; 1.0204x over previous
"""RNN-T Joiner kernel for 8 Trainium2 NeuronCores.

Reference computation (per batch element n):
    enc = encoder_out[n] @ W_enc.T + b_enc          # (T=200, J=512)
    dec = decoder_out[n] @ W_dec.T + b_dec          # (U=50,  J=512)
    x   = tanh(enc[:,None,:] + dec[None,:,:])       # (T, U, J)
    out = x @ W_out.T + b_out                       # (T, U, V=500)

Sharding: data-parallel over N=8 (one batch element per core).

Device-side dataflow (everything j/c-major, pre-transposed on host):
    PE:     encT/decT projections (fp32), main matmul (bf16, fp32 PSUM)
    GPSIMD: S[j,t,u] = encT[j,t] + decT[j,u]  (broadcast add, bf16 out)
    ACT:    XT = tanh(S)  (bf16)
    DVE:    PSUM -> SBUF evacuation + b_out bias (batched 4 M-tiles/instr)
    DMA:    1MB contiguous output stores
"""

import numpy as np

N, T, U = 8, 200, 50
C = 512   # enc/dec feature dim
J = 512   # joint dim
V = 500   # vocab
TU = T * U
P = 128
KC = J // P          # 4 contraction chunks of 128
T_BLK = 64           # t's per block (64*50 = 3200 tu = 25 M-tiles of 128)
BLK_TU = T_BLK * U   # 3200
GROUP = 4            # M-tiles per PSUM tile / output DMA group

_CACHE = {}


def _build_bass():
    import concourse.bass as bass  # noqa: F401
    import concourse.mybir as mybir
    import concourse.tile as tile
    from concourse import bacc

    bf16 = mybir.dt.bfloat16
    f32 = mybir.dt.float32
    Act = mybir.ActivationFunctionType

    nc = bacc.Bacc("TRN2", target_bir_lowering=False, debug=False, num_devices=N)

    enc_in = nc.dram_tensor("enc_in", [C, T], bf16, kind="ExternalInput").ap()
    dec_in = nc.dram_tensor("dec_in", [C, U], bf16, kind="ExternalInput").ap()
    w_enc = nc.dram_tensor("w_enc", [C, J], bf16, kind="ExternalInput").ap()
    w_dec = nc.dram_tensor("w_dec", [C, J], bf16, kind="ExternalInput").ap()
    w_out = nc.dram_tensor("w_out", [J, V], bf16, kind="ExternalInput").ap()
    b_enc = nc.dram_tensor("b_enc", [P, KC], f32, kind="ExternalInput").ap()
    b_dec = nc.dram_tensor("b_dec", [P, KC], f32, kind="ExternalInput").ap()
    b_out = nc.dram_tensor("b_out", [P, V], f32, kind="ExternalInput").ap()
    logits = nc.dram_tensor("logits", [TU, V], bf16, kind="ExternalOutput").ap()

    n_full = TU // P          # 78 full M-tiles
    tail = TU - n_full * P    # 16

    with tile.TileContext(nc) as tc:
        with (
            tc.tile_pool(name="const", bufs=1) as const,
            tc.tile_pool(name="s", bufs=8) as sp,
            tc.tile_pool(name="xt", bufs=12) as xtp,
            tc.tile_pool(name="lout", bufs=3) as lp,
            tc.tile_pool(name="ps", bufs=2, space="PSUM") as psp,
        ):
            # ---- load weights + inputs -------------------------------------
            w_enc_sb = const.tile([P, KC, J], bf16)
            w_dec_sb = const.tile([P, KC, J], bf16)
            w_out_sb = const.tile([P, KC, V], bf16)
            enc_in_sb = const.tile([P, KC, T], bf16)
            dec_in_sb = const.tile([P, KC, U], bf16)
            enc_in_r = enc_in.rearrange("(kc p) t -> p kc t", p=P)
            dec_in_r = dec_in.rearrange("(kc p) u -> p kc u", p=P)
            w_enc_r = w_enc.rearrange("(kc p) j -> p kc j", p=P)
            w_dec_r = w_dec.rearrange("(kc p) j -> p kc j", p=P)
            nc.sync.dma_start(enc_in_sb[:], enc_in_r)
            nc.sync.dma_start(dec_in_sb[:], dec_in_r)
            for kc in range(KC):
                nc.sync.dma_start(w_enc_sb[:, kc], w_enc_r[:, kc])
            for kc in range(KC):
                nc.sync.dma_start(w_dec_sb[:, kc], w_dec_r[:, kc])
            nc.sync.dma_start(w_out_sb[:], w_out.rearrange("(kc p) v -> p kc v", p=P))
            b_enc_sb = const.tile([P, KC], f32)
            nc.sync.dma_start(b_enc_sb[:], b_enc)
            b_dec_sb = const.tile([P, KC], f32)
            nc.sync.dma_start(b_dec_sb[:], b_dec)
            b_out_sb = const.tile([P, V], f32)
            nc.sync.dma_start(b_out_sb[:], b_out)

            # ---- input projections, directly in transposed (j-major) form --
            encT = const.tile([P, KC, T], f32)
            decT = const.tile([P, KC, U], f32)
            for jc in range(KC):
                ps = psp.tile([P, GROUP, 512], f32, tag="ps", name="pse")
                for kc in range(KC):
                    nc.tensor.matmul(
                        ps[:, 0, :T],
                        lhsT=w_enc_sb[:, kc, jc * P:(jc + 1) * P],
                        rhs=enc_in_sb[:, kc, :],
                        start=(kc == 0),
                        stop=(kc == KC - 1),
                    )
                nc.scalar.activation(
                    encT[:, jc, :], ps[:, 0, :T], Act.Identity,
                    bias=b_enc_sb[:, jc:jc + 1],
                )
                ps = psp.tile([P, GROUP, 512], f32, tag="ps", name="psd")
                for kc in range(KC):
                    nc.tensor.matmul(
                        ps[:, 0, :U],
                        lhsT=w_dec_sb[:, kc, jc * P:(jc + 1) * P],
                        rhs=dec_in_sb[:, kc, :],
                        start=(kc == 0),
                        stop=(kc == KC - 1),
                    )
                nc.scalar.activation(
                    decT[:, jc, :], ps[:, 0, :U], Act.Identity,
                    bias=b_dec_sb[:, jc:jc + 1],
                )

            # ---- XT block production (GPSIMD add -> ACT tanh) --------------
            # xts[block][kc] -> flattened [P, block_tu] bf16 AP
            xts = []
            t0 = 0
            blk = 0
            while t0 < T:
                nt = min(T_BLK, T - t0)
                row = []
                # sub-chunk the first block so matmuls can start early
                nsub = 4 if blk == 0 else 1
                step = nt // nsub
                kc_tiles = []
                for kc in range(KC):
                    s_full = sp.tile([P, T_BLK, U], bf16, tag="s", name=f"s{kc}")
                    x_full = xtp.tile([P, T_BLK, U], bf16, tag="xt", name=f"xt{kc}")
                    kc_tiles.append((s_full[:, :nt, :], x_full[:, :nt, :]))
                for sub in range(nsub):
                    lo, hi = sub * step, (sub + 1) * step
                    for kc in range(KC):
                        s, x = kc_tiles[kc]
                        eng = (nc.vector if (blk == 0 and kc % 2 == 0)
                               else nc.gpsimd)
                        eng.tensor_add(
                            s[:, lo:hi, :],
                            encT[:, kc, t0 + lo:t0 + hi, None]
                            .to_broadcast((P, hi - lo, U)),
                            decT[:, kc, None, :].to_broadcast((P, hi - lo, U)),
                        )
                        nc.scalar.activation(
                            x[:, lo:hi, :], s[:, lo:hi, :], Act.Tanh
                        )
                for kc in range(KC):
                    row.append(kc_tiles[kc][1].rearrange("p t u -> p (t u)"))
                xts.append(row)
                t0 += nt
                blk += 1

            # ---- main matmul over flat M-tile groups -----------------------
            def lhsT_for(kc, m_lo, rows):
                b = m_lo // BLK_TU
                off = m_lo - b * BLK_TU
                return xts[b][kc][:, off:off + rows]

            m = 0
            while m < n_full:
                g = min(GROUP, n_full - m)
                take_tail = (m + g == n_full) and tail and (g < GROUP)
                nsub = g + (1 if take_tail else 0)
                ps = psp.tile([P, GROUP, 512], f32, tag="ps", name="psm")
                L = lp.tile([P, GROUP, V], bf16, tag="L", name="L")
                for i in range(nsub):
                    rows = P if i < g else tail
                    for kc in range(KC):
                        nc.tensor.matmul(
                            ps[:rows, i, :V],
                            lhsT=lhsT_for(kc, (m + i) * P, rows),
                            rhs=w_out_sb[:, kc, :],
                            start=(kc == 0),
                            stop=(kc == KC - 1),
                        )
                nc.vector.tensor_add(
                    L[:, :nsub, :],
                    ps[:, :nsub, :V],
                    b_out_sb[:, None, :].to_broadcast((P, nsub, V)),
                )
                nc.sync.dma_start(
                    logits[m * P:(m + g) * P, :].rearrange("(i p) v -> p i v", p=P),
                    L[:, :g, :],
                )
                if take_tail:
                    nc.sync.dma_start(
                        logits[n_full * P:TU, :],
                        L[:tail, g, :],
                    )
                m += g

    nc.compile()
    return nc


def _get_bass():
    if "nc" not in _CACHE:
        _CACHE["nc"] = _build_bass()
    return _CACHE["nc"]


def _pack_inputs(inputs):
    import ml_dtypes

    encoder_out = np.ascontiguousarray(
        np.asarray(inputs["encoder_out"], np.float32).transpose(0, 2, 1)
        .astype(ml_dtypes.bfloat16))
    decoder_out = np.ascontiguousarray(
        np.asarray(inputs["decoder_out"], np.float32).transpose(0, 2, 1)
        .astype(ml_dtypes.bfloat16))
    WencT = np.ascontiguousarray(
        np.asarray(inputs["W_enc"], np.float32).T.astype(ml_dtypes.bfloat16))
    WdecT = np.ascontiguousarray(
        np.asarray(inputs["W_dec"], np.float32).T.astype(ml_dtypes.bfloat16))
    WoutT = np.ascontiguousarray(
        np.asarray(inputs["W_out"], np.float32).T.astype(ml_dtypes.bfloat16))
    benc = np.ascontiguousarray(
        np.asarray(inputs["b_enc"], np.float32).reshape(KC, P).T)
    bdec = np.ascontiguousarray(
        np.asarray(inputs["b_dec"], np.float32).reshape(KC, P).T)
    bout = np.ascontiguousarray(
        np.broadcast_to(np.asarray(inputs["b_out"], np.float32), (P, V)))
    return [
        {
            "enc_in": encoder_out[n],
            "dec_in": decoder_out[n],
            "w_enc": WencT,
            "w_dec": WdecT,
            "w_out": WoutT,
            "b_enc": benc,
            "b_dec": bdec,
            "b_out": bout,
        }
        for n in range(N)
    ]


def run(inputs, trace=False):
    """Run the bass kernel; returns (output array, BassKernelResults)."""
    from concourse.bass_utils import run_bass_kernel_spmd

    nc = _get_bass()
    in_maps = _pack_inputs(inputs)
    res = run_bass_kernel_spmd(nc, in_maps, core_ids=list(range(N)), trace=trace)
    out = np.stack([np.asarray(r["logits"], dtype=np.float32)
                    for r in res.results])
    return out.reshape(N, T, U, V), res


def kernel(**inputs):
    out, _ = run(inputs)
    return out
